# revision 15
# baseline (speedup 1.0000x reference)
"""CLAHE preprocessing layer - Trainium2 Bass kernel (8-core data-parallel).

Self-contained: builds and runs a Bass/Tile kernel implementing
  gray = round-half-even(0.299 R + 0.587 G + 0.114 B)   (uint8 input)
  per-tile (8x8 grid of 28x28) 256-bin histograms (PE nibble matmuls)
  CLAHE clip (limit 9) + uniform redistribution + cdf -> per-tile LUT
  bilinear 4-LUT interpolation per pixel -> uint8 gray output

Host side: floor/uint8 cast of the float input before upload, and
f32 x3-channel replication after download (both exact / within the
rounding budget), so only 38.6MB goes up and 12.8MB comes down the
axon tunnel instead of 154MB each way.

The sharded PJRT executables are AOT-compiled once and cached; repeat
calls skip tracing/lowering/NEFF-reload entirely. The batch runs as
pipelined chunks [96, 96, 32, 32]: the tunnel is duplex with shared
aggregate bandwidth, so big chunks go first (their large downloads
drain while later uploads stream) and small tail chunks minimize the
exposed exec+download+post tail. Host casts/posts and device exec
(~10.7ms per 2-image round per core) hide under the transfers; the
wall is the transport's aggregate-bandwidth floor (~51MB total).

Sharding: each call spreads its images evenly across the 8 cores.
"""
import numpy as np

import jax
import jax.numpy as jnp
from jax.sharding import Mesh, PartitionSpec, NamedSharding

try:
    from jax.experimental.shard_map import shard_map
except ImportError:  # newer jax
    from jax import shard_map

import concourse.bacc as bacc
import concourse.mybir as mybir
import concourse.tile as tile
from concourse.tile import add_dep_helper
from concourse import bass2jax

F32 = mybir.dt.float32
I16 = mybir.dt.int16
U8 = mybir.dt.uint8
BF16 = mybir.dt.bfloat16
AL = mybir.AluOpType

B_FULL = 256
N_CORES = 8
GRID = 8
TH = 28
AREA = TH * TH
PADAREA = 896
NB = 256
LIMIT = 9.0
TPI = GRID * GRID
H = W = GRID * TH


def frac_w(d):
    f = (d + 0.5) / TH - 0.5
    return float(f - np.floor(f))


def build_kernel(nc, n_img):
    x = nc.dram_tensor("x", [n_img, H, W, 3], U8, kind="ExternalInput")
    y = nc.dram_tensor("y", [n_img, H, W], U8, kind="ExternalOutput")
    hist_dram = nc.dram_tensor("hist_scratch", [16 * 128 * 16], F32, kind="Internal")
    lutcp_dram = nc.dram_tensor("lutcp", [2, GRID, 10, NB], F32, kind="Internal")

    ipr = 2
    T = ipr * TPI
    assert n_img % ipr == 0
    nrounds = n_img // ipr
    FULL_BLOCKS = AREA // 128
    TAIL = AREA - FULL_BLOCKS * 128
    NBLK = FULL_BLOCKS + 1

    with tile.TileContext(nc) as tc:
        with tc.tile_pool(name="const", bufs=1) as cpool, \
             tc.tile_pool(name="psum", bufs=2, space="PSUM") as ppool, \
             tc.tile_pool(name="work", bufs=1) as wpool, \
             tc.tile_pool(name="lutp", bufs=1) as lpool:
            iota_pl = cpool.tile([128, 16 * T], I16)
            nc.gpsimd.iota(iota_pl[:].rearrange("p (b t) -> p b t", b=16),
                           pattern=[[1, 16], [0, T]], base=0, channel_multiplier=0)
            iota_v = cpool.tile([128, NB], F32)
            nc.gpsimd.iota(iota_v[:], pattern=[[1, NB]], base=0, channel_multiplier=0,
                           allow_small_or_imprecise_dtypes=True)

            for r in range(nrounds):
                img0 = r * ipr
                # ---- load (TM layout, uint8) ----
                xt = wpool.tile([128, AREA * 3], U8, tag="xt")
                for i in range(ipr):
                    src = x.ap()[img0 + i].rearrange(
                        "(ty dy) (tx dx) c -> ty tx dy (dx c)", ty=GRID, tx=GRID)
                    for ty in range(GRID):
                        p0 = i * TPI + ty * GRID
                        nc.sync.dma_start(xt[p0:p0 + GRID, :], src[ty])

                # ---- gray = RNE(0.299 R + 0.587 G + 0.114 B) ----
                xf = wpool.tile([128, AREA * 3], F32, tag="xf")
                nc.vector.tensor_copy(xf[:], xt[:])
                xfv = xf[:].rearrange("p (a c) -> p a c", c=3)
                t0 = wpool.tile([128, AREA], F32, tag="t0")
                nc.vector.tensor_scalar(t0[:], xfv[:, :, 1], 0.587, None, op0=AL.mult)
                s1 = wpool.tile([128, AREA], F32, tag="s1")
                nc.vector.scalar_tensor_tensor(s1[:], in0=xfv[:, :, 0], scalar=0.299,
                                               in1=t0[:], op0=AL.mult, op1=AL.add)
                s2 = wpool.tile([128, AREA], F32, tag="s2")
                nc.vector.scalar_tensor_tensor(s2[:], in0=xfv[:, :, 2], scalar=0.114,
                                               in1=s1[:], op0=AL.mult, op1=AL.add)
                gi = wpool.tile([128, AREA], I16, tag="gi")
                nc.vector.tensor_copy(gi[:], s2[:])  # RNE cast = round half-even
                gray_f = wpool.tile([128, AREA], F32, tag="gray_f")
                nc.vector.tensor_copy(gray_f[:], gi[:])

                h_tm = wpool.tile([128, PADAREA], I16, tag="h_tm")
                l_tm = wpool.tile([128, PADAREA], I16, tag="l_tm")
                nc.vector.tensor_scalar(h_tm[:, :AREA], gi[:], 4, None,
                                        op0=AL.logical_shift_right)
                nc.vector.tensor_scalar(l_tm[:, :AREA], gi[:], 15, None,
                                        op0=AL.bitwise_and)
                nc.vector.memset(h_tm[:, AREA:], 0)
                nc.vector.memset(l_tm[:, AREA:], 0)

                # ---- transpose to PMT ----
                h_pm = wpool.tile([128, NBLK * 128], I16, tag="h_pm")
                l_pm = wpool.tile([128, NBLK * 128], I16, tag="l_pm")
                for k in range(NBLK):
                    nc.sync.dma_start_transpose(
                        h_pm[:, k * 128:k * 128 + T], h_tm[:T, k * 128:(k + 1) * 128])
                    nc.sync.dma_start_transpose(
                        l_pm[:, k * 128:k * 128 + T], l_tm[:T, k * 128:(k + 1) * 128])

                # ---- one-hots + hist matmuls ----
                hist_ps = ppool.tile([128, T * 16], F32, space="PSUM", tag="hist_ps")
                ohh_all = wpool.tile([128, NBLK * 16 * T], BF16, tag="ohh_all")
                ohl_all = wpool.tile([128, NBLK * 16 * T], BF16, tag="ohl_all")
                for k in range(NBLK):
                    nc.vector.tensor_tensor(
                        ohh_all[:, k * 16 * T:(k + 1) * 16 * T]
                        .rearrange("p (b t) -> p b t", b=16),
                        h_pm[:, k * 128:k * 128 + T]
                        .rearrange("p (o t) -> p o t", o=1).to_broadcast([128, 16, T]),
                        iota_pl[:].rearrange("p (b t) -> p b t", b=16), op=AL.is_equal)
                    nc.vector.tensor_tensor(
                        ohl_all[:, k * 16 * T:(k + 1) * 16 * T]
                        .rearrange("p (b t) -> p b t", b=16),
                        l_pm[:, k * 128:k * 128 + T]
                        .rearrange("p (o t) -> p o t", o=1).to_broadcast([128, 16, T]),
                        iota_pl[:].rearrange("p (b t) -> p b t", b=16), op=AL.is_equal)
                for t in range(T):
                    for k in range(NBLK):
                        nparts = 128 if k < FULL_BLOCKS else TAIL
                        base = k * 16 * T
                        lhsT = ohh_all[:nparts, base:base + 16 * T] \
                            .rearrange("p (b tt) -> p tt b", tt=T)[:, t]
                        rhs = ohl_all[:nparts, base:base + 16 * T] \
                            .rearrange("p (b tt) -> p tt b", tt=T)[:, t]
                        nc.tensor.matmul(
                            hist_ps[0:16, t * 16:t * 16 + 16],
                            lhsT=lhsT, rhs=rhs,
                            start=(k == 0), stop=(k == NBLK - 1))

                # ---- hist -> SBUF TM + LUT build ----
                hist_flat = lpool.tile([16, T * 16], F32, tag="hist_flat")
                nc.vector.tensor_copy(hist_flat[:], hist_ps[0:16])
                hw_i = nc.sync.dma_start(hist_dram.ap(), hist_flat[:])
                hist_sb = lpool.tile([128, NB], F32, tag="hist_sb")
                hr_i = nc.sync.dma_start(
                    hist_sb[:].rearrange("t (h l) -> t h l", h=16),
                    hist_dram.ap().rearrange("(h t l) -> t h l", h=16, t=T))
                add_dep_helper(hr_i.ins, hw_i.ins, reason="hist dram RAW")

                clip_t = lpool.tile([128, NB], F32, tag="clip_t")
                nc.vector.tensor_scalar(clip_t[:], hist_sb[:], LIMIT, None, op0=AL.min)
                ssum = lpool.tile([128, 1], F32, tag="ssum")
                nc.vector.tensor_reduce(ssum[:], clip_t[:],
                                        axis=mybir.AxisListType.X, op=AL.add)
                alpha = lpool.tile([128, 1], F32, tag="alpha")
                nc.vector.tensor_scalar(alpha[:], ssum[:], -1.0 / NB, AREA / NB,
                                        op0=AL.mult, op1=AL.add)
                # clip2 = clipped + excess/NB (exact reference order), then cumsum
                clip2 = lpool.tile([128, NB], F32, tag="clip2")
                nc.vector.tensor_scalar(clip2[:], clip_t[:], alpha[:, :1], None,
                                        op0=AL.add)
                S = lpool.tile([128, NB], F32, tag="S")
                zz = lpool.tile([128, NB], F32, tag="zz")
                nc.vector.memset(zz[:], 0.0)
                nc.vector.tensor_tensor_scan(S[:], data0=clip2[:], data1=zz[:],
                                             initial=0.0, op0=AL.add, op1=AL.add)
                lutf = lpool.tile([128, NB], F32, tag="lutf")
                nc.vector.tensor_scalar(lutf[:], S[:], 255.0 / AREA, None,
                                        op0=AL.mult)
                luti = lpool.tile([128, NB], I16, tag="luti")
                nc.vector.tensor_copy(luti[:], lutf[:])
                lut = lpool.tile([128, NB], F32, tag="lut")
                nc.vector.tensor_copy(lut[:], luti[:])

                # ---- LUT9 via col-padded DRAM ----
                pad_writes = []
                w1 = nc.sync.dma_start(lutcp_dram.ap()[:, :, 1:9], lut[:])
                pad_writes.append(w1)
                tmp16 = lpool.tile([16, 2 * NB], F32, tag="tmp16")
                r1 = nc.sync.dma_start(
                    tmp16[:, :NB],
                    lutcp_dram.ap()[:, :, 1].rearrange("i ty b -> (i ty) b"))
                add_dep_helper(r1.ins, w1.ins, reason="padcol RAW")
                r2 = nc.sync.dma_start(
                    tmp16[:, NB:],
                    lutcp_dram.ap()[:, :, 8].rearrange("i ty b -> (i ty) b"))
                add_dep_helper(r2.ins, w1.ins, reason="padcol RAW")
                w2 = nc.sync.dma_start(
                    lutcp_dram.ap()[:, :, 0].rearrange("i ty b -> (i ty) b"),
                    tmp16[:, :NB])
                pad_writes.append(w2)
                w3 = nc.sync.dma_start(
                    lutcp_dram.ap()[:, :, 9].rearrange("i ty b -> (i ty) b"),
                    tmp16[:, NB:])
                pad_writes.append(w3)

                lut9 = lpool.tile([128, 9 * NB], F32, tag="lut9")
                l9v = lut9[:].rearrange("p (s c b) -> p s c b", s=3, c=3)

                def g_dep(gi_):
                    for pw in pad_writes:
                        add_dep_helper(gi_.ins, pw.ins, reason="lutpad RAW")

                cpa = lutcp_dram.ap()
                for sidx in range(3):
                    for cidx in range(3):
                        if sidx == 1:
                            g_dep(nc.sync.dma_start(
                                l9v[:, sidx, cidx], cpa[:, :, cidx:cidx + GRID]))
                        else:
                            for i in range(ipr):
                                p0 = i * TPI
                                if sidx == 0:
                                    g_dep(nc.sync.dma_start(
                                        l9v[p0:p0 + GRID, sidx, cidx],
                                        cpa[i, 0:1, cidx:cidx + GRID]))
                                    g_dep(nc.sync.dma_start(
                                        l9v[p0 + GRID:p0 + TPI, sidx, cidx],
                                        cpa[i, 0:GRID - 1, cidx:cidx + GRID]))
                                else:
                                    g_dep(nc.sync.dma_start(
                                        l9v[p0:p0 + TPI - GRID, sidx, cidx],
                                        cpa[i, 1:GRID, cidx:cidx + GRID]))
                                    g_dep(nc.sync.dma_start(
                                        l9v[p0 + TPI - GRID:p0 + TPI, sidx, cidx],
                                        cpa[i, GRID - 1:GRID, cidx:cidx + GRID]))

                # ---- BLx + per-slot lookups + y blend ----
                blx = lpool.tile([128, 2 * TH * NB], F32, tag="blx")
                blxv = blx[:].rearrange("p (s d b) -> p s d b", s=2, d=TH)

                def build_blx(slot, s):
                    for dx in range(TH):
                        wxv = frac_w(dx)
                        cL, cR = (0, 1) if dx < TH // 2 else (1, 2)
                        nc.vector.tensor_scalar(blxv[:, slot, dx], l9v[:, s, cL],
                                                1.0 - wxv, None, op0=AL.mult)
                        nc.vector.scalar_tensor_tensor(
                            blxv[:, slot, dx], in0=l9v[:, s, cR], scalar=wxv,
                            in1=blxv[:, slot, dx], op0=AL.mult, op1=AL.add)

                build_blx(0, 0)
                build_blx(1, 1)

                o0 = wpool.tile([128, AREA], F32, tag="o0")
                o1 = wpool.tile([128, AREA], F32, tag="o1")
                scr = wpool.tile([128, NB], F32, tag="scr")
                scr2 = scr
                for dy in range(TH // 2):
                    for dx in range(TH):
                        j = dy * TH + dx
                        g_col = gray_f[:, j:j + 1]
                        nc.vector.scalar_tensor_tensor(
                            scr[:], in0=iota_v[:], scalar=g_col,
                            in1=blxv[:, 0, dx], op0=AL.is_equal, op1=AL.mult,
                            accum_out=o0[:, j:j + 1])
                        nc.vector.scalar_tensor_tensor(
                            scr2[:], in0=iota_v[:], scalar=g_col,
                            in1=blxv[:, 1, dx], op0=AL.is_equal, op1=AL.mult,
                            accum_out=o1[:, j:j + 1])
                build_blx(0, 2)
                for dy in range(TH // 2, TH):
                    for dx in range(TH):
                        j = dy * TH + dx
                        g_col = gray_f[:, j:j + 1]
                        nc.vector.scalar_tensor_tensor(
                            scr[:], in0=iota_v[:], scalar=g_col,
                            in1=blxv[:, 1, dx], op0=AL.is_equal, op1=AL.mult,
                            accum_out=o0[:, j:j + 1])
                        nc.vector.scalar_tensor_tensor(
                            scr2[:], in0=iota_v[:], scalar=g_col,
                            in1=blxv[:, 0, dx], op0=AL.is_equal, op1=AL.mult,
                            accum_out=o1[:, j:j + 1])

                out_tm = wpool.tile([128, AREA], F32, tag="out_tm")
                t01 = wpool.tile([128, AREA], F32, tag="t01")
                ov = out_tm[:].rearrange("p (dy dx) -> p dy dx", dy=TH)
                tv = t01[:].rearrange("p (dy dx) -> p dy dx", dy=TH)
                o0v = o0[:].rearrange("p (dy dx) -> p dy dx", dy=TH)
                o1v = o1[:].rearrange("p (dy dx) -> p dy dx", dy=TH)
                for dy in range(TH):
                    wyv = frac_w(dy)
                    nc.vector.tensor_scalar(tv[:, dy], o0v[:, dy], 1.0 - wyv, None,
                                            op0=AL.mult)
                    nc.vector.scalar_tensor_tensor(
                        ov[:, dy], in0=o1v[:, dy], scalar=wyv, in1=tv[:, dy],
                        op0=AL.mult, op1=AL.add)

                # ---- store (uint8 gray, single channel) ----
                out_u8 = wpool.tile([128, AREA], U8, tag="out_u8")
                nc.vector.tensor_copy(out_u8[:], out_tm[:])  # RNE, in [0,255]
                for i in range(ipr):
                    dst = y.ap()[img0 + i].rearrange(
                        "(ty dy) (tx dx) -> ty tx dy dx", ty=GRID, tx=GRID)
                    for ty in range(GRID):
                        p0 = i * TPI + ty * GRID
                        nc.sync.dma_start(dst[ty], out_u8[p0:p0 + GRID])
    return x, y


class _Runner:
    """AOT-compiles the sharded PJRT executable once for a fixed
    per-call batch (`chunk` images over 8 cores) and reuses it."""

    def __init__(self, chunk):
        self.chunk = chunk
        nc = bacc.Bacc("TRN2", target_bir_lowering=False, num_devices=N_CORES)
        build_kernel(nc, chunk // N_CORES)
        nc.compile()
        bass2jax.install_neuronx_cc_hook()

        partition_name = (nc.partition_id_tensor.name
                          if nc.partition_id_tensor else None)
        in_names, out_names, out_avals = [], [], []
        for alloc in nc.m.functions[0].allocations:
            if not isinstance(alloc, mybir.MemoryLocationSet):
                continue
            name = alloc.memorylocations[0].name
            if alloc.kind == "ExternalInput":
                if name != partition_name:
                    in_names.append(name)
            elif alloc.kind == "ExternalOutput":
                out_names.append(name)
                out_avals.append(jax.core.ShapedArray(
                    tuple(alloc.tensor_shape), mybir.dt.np(alloc.dtype)))
        n_params = len(in_names)
        n_outs = len(out_avals)
        in_names_all = in_names + out_names + (
            [partition_name] if partition_name else [])
        donate = tuple(range(n_params, n_params + n_outs))

        def _body(*args):
            operands = list(args)
            if partition_name is not None:
                operands.append(bass2jax.partition_id_tensor())
            outs = bass2jax._bass_exec_p.bind(
                *operands,
                out_avals=tuple(out_avals), in_names=tuple(in_names_all),
                out_names=tuple(out_names),
                lowering_input_output_aliases=(),
                sim_require_finite=True, sim_require_nnan=True, nc=nc)
            return tuple(outs)

        devices = jax.devices()[:N_CORES]
        self.mesh = Mesh(np.asarray(devices), ("core",))
        self.sharding = NamedSharding(self.mesh, PartitionSpec("core"))
        in_specs = (PartitionSpec("core"),) * (n_params + n_outs)
        out_specs = (PartitionSpec("core"),) * n_outs

        x_spec = jax.ShapeDtypeStruct((chunk, H, W, 3), np.uint8)
        z_spec = jax.ShapeDtypeStruct((chunk, H, W), np.uint8)
        self.compiled = bass2jax.fast_dispatch_compile(lambda: jax.jit(
            shard_map(_body, mesh=self.mesh, in_specs=in_specs,
                      out_specs=out_specs, check_rep=False),
            donate_argnums=donate, keep_unused=True,
        ).lower(x_spec, z_spec).compile())

    def start(self, x_u8_chunk):
        """Dispatch one chunk (upload starts async); returns the jax array."""
        zeros = jnp.zeros((self.chunk, H, W), jnp.uint8, device=self.sharding)
        (y,) = self.compiled(x_u8_chunk, zeros)
        y.copy_to_host_async()
        return y


_RUNNERS = {}
_OUT_BUF = None
_POOL = None
_U8_BUFS = {}


def _u8_buf(key, shape):
    """Reusable pre-touched uint8 staging buffer (one per chunk slot)."""
    buf = _U8_BUFS.get(key)
    if buf is None or buf.shape != shape:
        buf = np.empty(shape, np.uint8)
        buf.fill(0)
        _U8_BUFS[key] = buf
    return buf


def _get_runner(chunk):
    global _OUT_BUF, _POOL
    if chunk not in _RUNNERS:
        from concurrent.futures import ThreadPoolExecutor
        _RUNNERS[chunk] = _Runner(chunk)
        if _POOL is None:
            _POOL = ThreadPoolExecutor(8)
    if _OUT_BUF is None or _OUT_BUF.shape[0] != B_FULL:
        _OUT_BUF = np.empty((B_FULL, H, W, 3), np.float32)
        _OUT_BUF.fill(0.0)  # pre-touch pages
    return _RUNNERS[chunk]


def _post_chunk(out, g, off):
    """Write uint8 gray chunk into the f32 x3-channel output, threaded."""
    n = g.shape[0]
    step = max(1, n // 8)

    def _p(k):
        s0, s1 = k * step, min((k + 1) * step, n)
        if s0 < s1:
            out[off + s0:off + s1] = g[s0:s1, :, :, None]

    return list(_POOL.map(_p, range((n + step - 1) // step)))


def _chunk_plan(b):
    """Pipelined chunk sizes. The tunnel is duplex with SHARED aggregate
    bandwidth: big chunks first let their large downloads start draining
    while later uploads stream; small tail chunks minimize the exposed
    exec+download+post tail. Per-core image count (chunk/8) must be even."""
    if b == 256:
        return [96, 96, 32, 32]
    if b % (2 * N_CORES) == 0 and b // 2 % (2 * N_CORES) == 0:
        return [b // 2, b // 2]
    return [b]


def kernel(x):
    """x: [256, 224, 224, 3] float32 -> [256, 224, 224, 3] float32."""
    x = np.asarray(x)
    b = x.shape[0]
    plan = _chunk_plan(b)
    runners = {n: _get_runner(n) for n in set(plan)}
    global _OUT_BUF
    if _OUT_BUF.shape[0] != b:
        _OUT_BUF = np.empty((b, H, W, 3), np.float32)
        _OUT_BUF.fill(0.0)
    out = _OUT_BUF

    ys = []
    off = 0
    for slot, n in enumerate(plan):
        # floor for non-negative == C-cast truncation; input is [0, 255)
        x_u8 = _u8_buf(slot, (n, H, W, 3))
        np.copyto(x_u8, x[off:off + n], casting="unsafe")
        ys.append((off, runners[n].start(x_u8)))
        off += n

    futs = []
    for off, y in ys:
        g = np.asarray(y)  # blocks until this chunk's download completes
        futs.append(_POOL.submit(_post_chunk, out, g, off))
    for f in futs:
        f.result()
    return out
